# revision 28
# baseline (speedup 1.0000x reference)
"""ConvGRU Trainium2 kernel.

video [B=2, T=16, C=128, H=64, W=64] f32; 1x1-conv GRU over T.
Sharding: data-parallel over (B x H/16) -> 8 cores, each core owns
P = 16*64 = 1024 pixels for all T; weights replicated.

Per core, per timestep (pixels on the free dim, channels on partitions):
    zr_pre = [Wzx@x + Wzh@h | Wrx@x + Wrh@h]      (PE, fp16 in / fp32 psum)
    z = sigmoid(zr_pre[:P] + bz); r = sigmoid(zr_pre[P:] + br)   (ACT)
    rh = r * h                                     (DVE)
    c = tanh(Whx@x + Whh@rh + bh)                  (PE + ACT)
    h = h + z * (c - h)                            (DVE, fp16 state)

The recurrence is latency-bound: each pixel group's step is a serial
cross-engine chain (h -> Wrh matmul -> sigmoid -> r*h -> Whh matmul ->
tanh -> blend -> h').  Structure choices below all serve that chain:
  - x-side matmul contributions for step t+1 are issued into step t's
    tail (PSUM c-tiles double buffered) to keep the PE dense/warm
  - r-gate work goes first (it gates the tanh matmul); z sigmoids are
    slotted late (only needed by the final blend)
  - group priority alternates per step so the second group's queueing
    penalty averages out instead of compounding on one chain
  - warmup matmuls + an early dummy activation hide the HAM clock-gate
    ramp and the ACT table load behind the initial x DMA

Numerics: fp16 matmul inputs/gates/state, fp32 PSUM accum + fp32 bias.
"""

import os
import sys

import numpy as np

B, T, C, H, W = 2, 16, 128, 64, 64
NCORES = 8
HQ = H // 4          # 16 rows of H per core (4 H-slices x 2 batches = 8 cores)
P = HQ * W           # 1024 pixels per core
G = 2                # pixel groups per step (independent recurrence chains)
PG = P // G          # 512 pixels per group

_PROG = None


def _ensure_paths():
    for p in ("/opt/trn_rl_repo",):
        if p not in sys.path and os.path.isdir(p):
            sys.path.append(p)


def _build():
    _ensure_paths()
    import concourse.bacc as bacc
    import concourse.tile as tile
    from concourse import mybir

    f32 = mybir.dt.float32
    f16 = mybir.dt.float16
    AF = mybir.ActivationFunctionType

    nc = bacc.Bacc(
        "TRN2", target_bir_lowering=False, debug=False, num_devices=NCORES
    )
    x_dram = nc.dram_tensor("x_seq", [T, C, P], f16, kind="ExternalInput")
    w_dram = nc.dram_tensor("wmats", [C, 6 * C], f16, kind="ExternalInput")
    b_dram = nc.dram_tensor("biases", [C, 4], f32, kind="ExternalInput")
    o_dram = nc.dram_tensor("out_seq", [T, C, P], f16, kind="ExternalOutput")

    x_ap = x_dram.ap()
    w_ap = w_dram.ap()
    b_ap = b_dram.ap()
    o_ap = o_dram.ap()

    WZX, WZH, WRX, WRH, WHX, WHH = range(6)

    with tile.TileContext(nc) as tc:
        with (
            tc.tile_pool(name="consts", bufs=1) as consts,
            tc.tile_pool(name="xin", bufs=4) as xpool,
            tc.tile_pool(name="state", bufs=2) as spool,
            tc.tile_pool(name="work", bufs=2) as wk,
            tc.tile_pool(name="ps", bufs=1, space="PSUM") as ps,
        ):
            # garbage tile: warmup matmuls need no DMA, so the PE ramp and
            # ACT table load start immediately and overlap the input DMAs
            junk = consts.tile([C, PG], f16)
            nc.vector.memset(junk[:], 0.0)

            wt = consts.tile([C, 6 * C], f16)
            nc.scalar.dma_start(wt[:], w_ap[:])
            bt = consts.tile([C, 4], f32)
            nc.gpsimd.dma_start(bt[:], b_ap[:])

            def wslice(i):
                return wt[:, i * C : (i + 1) * C]

            h16 = [None] * G

            # -- warmup: ramp the PE clock gate + preload the ACT table
            #    while the first x DMA is in flight.  Aliases the c_0 tag
            #    (bufs=2) so t0's single-buffered zr openers don't WAW-wait
            #    on the warmup sigmoid's read. --
            warm = ps.tile([C, PG], f32, tag="c_0", bufs=2)
            for i in range(3):
                nc.tensor.matmul(
                    warm[:], junk[:, :C], junk[:], start=True, stop=True
                )
            wtmp = wk.tile([C, PG], f16, tag="r_0")
            nc.scalar.activation(
                wtmp[:, :C], warm[:, :C], AF.Sigmoid, bias=0.0
            )

            def load_x(t):
                xt = xpool.tile([C, P], f16, tag="x")
                nc.sync.dma_start(xt[:], x_ap[t])
                return xt

            def open_zr(xt, gorder):
                """Open z|r accumulations with the x-side contributions."""
                zr_t = [None] * G
                for g in gorder:
                    xs = xt[:, g * PG : (g + 1) * PG]
                    zr = ps.tile([C, 2 * PG], f32, tag=f"zr_{g}", bufs=1)
                    nc.tensor.matmul(
                        zr[:, PG:], wslice(WRX), xs, start=True, stop=False
                    )
                    nc.tensor.matmul(
                        zr[:, :PG], wslice(WZX), xs, start=True, stop=False
                    )
                    zr_t[g] = zr
                return zr_t

            def open_c(xt, gorder):
                cp_t = [None] * G
                for g in gorder:
                    xs = xt[:, g * PG : (g + 1) * PG]
                    cp = ps.tile([C, PG], f32, tag=f"c_{g}", bufs=2)
                    nc.tensor.matmul(
                        cp[:], wslice(WHX), xs, start=True, stop=False
                    )
                    cp_t[g] = cp
                return cp_t

            first = list(range(G))
            x_t = load_x(0)
            x_next = load_x(1)

            # -- t = 0 fast path (h == 0): only z and c are needed --
            #    z0 = sigmoid(Wzx@x0 + bz); c0 = tanh(Whx@x0 + bh);
            #    h1 = z0 * c0.  No r work, no h-closes, no blend adds.
            zr_t = [None] * G
            cp_t = [None] * G
            for g in first:
                xs = x_t[:, g * PG : (g + 1) * PG]
                zr = ps.tile([C, 2 * PG], f32, tag=f"zr_{g}", bufs=1)
                nc.tensor.matmul(
                    zr[:, :PG], wslice(WZX), xs, start=True, stop=True
                )
                zr_t[g] = zr
                cp = ps.tile([C, PG], f32, tag=f"c_{g}", bufs=2)
                nc.tensor.matmul(cp[:], wslice(WHX), xs, start=True, stop=True)
                cp_t[g] = cp

            for g in first:
                zt = wk.tile([C, PG], f16, tag=f"z_{g}")
                nc.scalar.activation(
                    zt[:], zr_t[g][:, :PG], AF.Sigmoid, bias=bt[:, 0:1]
                )
                ct = wk.tile([C, PG], f16, tag=f"c16_{g}")
                nc.scalar.activation(ct[:], cp_t[g][:], AF.Tanh, bias=bt[:, 2:3])
                n16 = spool.tile([C, PG], f16, tag=f"h16_{g}")
                nc.vector.tensor_mul(n16[:], zt[:], ct[:])
                h16[g] = n16
                nc.gpsimd.dma_start(o_ap[0, :, g * PG : (g + 1) * PG], n16[:])

            # openers for t = 1
            x_t = x_next
            zr_t = open_zr(x_t, first)
            cp_t = open_c(x_t, first)

            for t in range(1, T):
                go = first if t % 2 == 0 else first[::-1]
                x_next = load_x(t + 1) if t + 1 < T else None

                # -- PE: close the r then z accumulations (chain head) --
                for g in go:
                    nc.tensor.matmul(
                        zr_t[g][:, PG:], wslice(WRH), h16[g][:],
                        start=False, stop=True,
                    )
                for g in go:
                    nc.tensor.matmul(
                        zr_t[g][:, :PG], wslice(WZH), h16[g][:],
                        start=False, stop=True,
                    )

                # -- ACT: r sigmoids first (they gate rh -> c matmul) --
                r16 = [None] * G
                for g in go:
                    rt = wk.tile([C, PG], f16, tag=f"r_{g}")
                    nc.scalar.activation(
                        rt[:], zr_t[g][:, PG:], AF.Sigmoid, bias=bt[:, 1:2]
                    )
                    r16[g] = rt

                rh16 = [None] * G
                for g in go:
                    rh = wk.tile([C, PG], f16, tag=f"rh_{g}")
                    nc.vector.tensor_mul(rh[:], r16[g][:], h16[g][:])
                    rh16[g] = rh

                for g in go:
                    nc.tensor.matmul(
                        cp_t[g][:], wslice(WHH), rh16[g][:],
                        start=False, stop=True,
                    )

                # next step's c openers can run any time (double-buffered)
                cp_next = open_c(x_next, go) if x_next is not None else None

                # -- ACT: zbar/tanh interleaved; zbar = 1-z = sigmoid(-pre)
                #    feeds the blend h' = zbar*h + (1-zbar)*c, whose only
                #    post-tanh serial ops are v = z*c and h' = u + v --
                zb16, c16 = [None] * G, [None] * G
                for g in go:
                    zbt = wk.tile([C, PG], f16, tag=f"zb_{g}")
                    nc.scalar.activation(
                        zbt[:], zr_t[g][:, :PG], AF.Sigmoid,
                        bias=bt[:, 3:4], scale=-1.0,
                    )
                    zb16[g] = zbt
                    ct = wk.tile([C, PG], f16, tag=f"c16_{g}")
                    nc.scalar.activation(
                        ct[:], cp_t[g][:], AF.Tanh, bias=bt[:, 2:3]
                    )
                    c16[g] = ct

                # next step's z|r openers (wait on this step's sigmoids)
                zr_next = open_zr(x_next, go) if x_next is not None else None

                # -- DVE mid-chain: u = zbar*h and z = 1-zbar overlap the
                #    tanh; only v and the final add trail it --
                u16, z16 = [None] * G, [None] * G
                for g in go:
                    ut = wk.tile([C, PG], f16, tag=f"u_{g}")
                    nc.vector.tensor_mul(ut[:], zb16[g][:], h16[g][:])
                    u16[g] = ut
                    zt = wk.tile([C, PG], f16, tag=f"z_{g}")
                    nc.vector.tensor_scalar(
                        zt[:], zb16[g][:], -1.0, 1.0,
                        mybir.AluOpType.mult, mybir.AluOpType.add,
                    )
                    z16[g] = zt

                for g in go:
                    v16 = wk.tile([C, PG], f16, tag=f"v_{g}")
                    nc.vector.tensor_mul(v16[:], z16[g][:], c16[g][:])
                    n16 = spool.tile([C, PG], f16, tag=f"h16_{g}")
                    nc.vector.tensor_add(n16[:], u16[g][:], v16[:])
                    h16[g] = n16
                    nc.gpsimd.dma_start(
                        o_ap[t, :, g * PG : (g + 1) * PG], n16[:]
                    )

                if x_next is not None:
                    x_t, zr_t, cp_t = x_next, zr_next, cp_next

    nc.compile()
    return nc


def _get_prog():
    global _PROG
    if _PROG is None:
        _PROG = _build()
    return _PROG


def _make_in_maps(video, Wz, bz, Wr, br, Wh, bh):
    w6 = np.concatenate(
        [
            Wz[:, :C].T, Wz[:, C:].T,
            Wr[:, :C].T, Wr[:, C:].T,
            Wh[:, :C].T, Wh[:, C:].T,
        ],
        axis=1,
    ).astype(np.float16)
    b3 = np.stack([bz, br, bh, -bz], axis=1).astype(np.float32)
    in_maps = []
    for core in range(NCORES):
        b_, q = divmod(core, 4)
        xs = np.ascontiguousarray(
            video[b_, :, :, q * HQ : (q + 1) * HQ, :]
        ).reshape(T, C, P).astype(np.float16)
        in_maps.append({"x_seq": xs, "wmats": w6, "biases": b3})
    return in_maps


def kernel(video, Wz, bz, Wr, br, Wh, bh):
    _ensure_paths()
    from concourse.bass_utils import run_bass_kernel_spmd

    video = np.asarray(video, dtype=np.float32)
    nc = _get_prog()
    in_maps = _make_in_maps(video, Wz, bz, Wr, br, Wh, bh)
    res = run_bass_kernel_spmd(nc, in_maps, list(range(NCORES)))

    out = np.empty((B, T, C, H, W), np.float32)
    for core in range(NCORES):
        b_, q = divmod(core, 4)
        out[b_, :, :, q * HQ : (q + 1) * HQ, :] = np.asarray(
            res.results[core]["out_seq"]
        ).astype(np.float32).reshape(T, C, HQ, W)
    return out



# revision 29
# speedup vs baseline: 1.2018x; 1.2018x over previous
"""ConvGRU Trainium2 kernel.

video [B=2, T=16, C=128, H=64, W=64] f32; 1x1-conv GRU over T.
Sharding: data-parallel over (B x H/16) -> 8 cores, each core owns
P = 16*64 = 1024 pixels for all T; weights replicated.

Per core, per timestep (pixels on the free dim, channels on partitions):
    zr_pre = [Wzx@x + Wzh@h | Wrx@x + Wrh@h]      (PE, fp16 in / fp32 psum)
    z = sigmoid(zr_pre[:P] + bz); r = sigmoid(zr_pre[P:] + br)   (ACT)
    rh = r * h                                     (DVE)
    c = tanh(Whx@x + Whh@rh + bh)                  (PE + ACT)
    h = h + z * (c - h)                            (DVE, fp16 state)

The recurrence is latency-bound: each pixel group's step is a serial
cross-engine chain (h -> Wrh matmul -> sigmoid -> r*h -> Whh matmul ->
tanh -> blend -> h').  Structure choices below all serve that chain:
  - x-side matmul contributions for step t+1 are issued into step t's
    tail (PSUM c-tiles double buffered) to keep the PE dense/warm
  - r-gate work goes first (it gates the tanh matmul); z sigmoids are
    slotted late (only needed by the final blend)
  - group priority alternates per step so the second group's queueing
    penalty averages out instead of compounding on one chain
  - warmup matmuls + an early dummy activation hide the HAM clock-gate
    ramp and the ACT table load behind the initial x DMA

Numerics: fp16 matmul inputs/gates/state, fp32 PSUM accum + fp32 bias.
"""

import os
import sys

import numpy as np

B, T, C, H, W = 2, 16, 128, 64, 64
NCORES = 8
HQ = H // 4          # 16 rows of H per core (4 H-slices x 2 batches = 8 cores)
P = HQ * W           # 1024 pixels per core
G = 2                # pixel groups per step (independent recurrence chains)
PG = P // G          # 512 pixels per group

_PROG = None


def _ensure_paths():
    for p in ("/opt/trn_rl_repo",):
        if p not in sys.path and os.path.isdir(p):
            sys.path.append(p)


def _build():
    _ensure_paths()
    import concourse.bacc as bacc
    import concourse.tile as tile
    from concourse import mybir

    f32 = mybir.dt.float32
    f16 = mybir.dt.float16
    AF = mybir.ActivationFunctionType

    nc = bacc.Bacc(
        "TRN2", target_bir_lowering=False, debug=False, num_devices=NCORES
    )
    x_dram = nc.dram_tensor("x_seq", [T, C, P], f16, kind="ExternalInput")
    w_dram = nc.dram_tensor("wmats", [C, 6 * C], f16, kind="ExternalInput")
    b_dram = nc.dram_tensor("biases", [C, 4], f32, kind="ExternalInput")
    o_dram = nc.dram_tensor("out_seq", [T, C, P], f16, kind="ExternalOutput")

    x_ap = x_dram.ap()
    w_ap = w_dram.ap()
    b_ap = b_dram.ap()
    o_ap = o_dram.ap()

    WZX, WZH, WRX, WRH, WHX, WHH = range(6)

    with tile.TileContext(nc) as tc:
        with (
            tc.tile_pool(name="consts", bufs=1) as consts,
            tc.tile_pool(name="xin", bufs=4) as xpool,
            tc.tile_pool(name="state", bufs=2) as spool,
            tc.tile_pool(name="work", bufs=2) as wk,
            tc.tile_pool(name="ps", bufs=1, space="PSUM") as ps,
        ):
            # garbage tile: warmup matmuls need no DMA, so the PE ramp and
            # ACT table load start immediately and overlap the input DMAs
            junk = consts.tile([C, PG], f16)
            nc.vector.memset(junk[:], 0.0)

            wt = consts.tile([C, 6 * C], f16)
            nc.scalar.dma_start(wt[:], w_ap[:])
            bt = consts.tile([C, 4], f32)
            nc.gpsimd.dma_start(bt[:], b_ap[:])

            def wslice(i):
                return wt[:, i * C : (i + 1) * C]

            h16 = [None] * G

            # -- warmup: ramp the PE clock gate + preload the ACT table
            #    while the first x DMA is in flight.  Aliases the c_0 tag
            #    (bufs=2) so t0's single-buffered zr openers don't WAW-wait
            #    on the warmup sigmoid's read. --
            warm = ps.tile([C, PG], f32, tag="c_0", bufs=2)
            for i in range(3):
                nc.tensor.matmul(
                    warm[:], junk[:, :C], junk[:], start=True, stop=True
                )
            wtmp = wk.tile([C, PG], f16, tag="r_0")
            nc.scalar.activation(
                wtmp[:, :C], warm[:, :C], AF.Sigmoid, bias=0.0
            )

            def load_x(t):
                xt = xpool.tile([C, P], f16, tag="x")
                nc.sync.dma_start(xt[:], x_ap[t])
                return xt

            def open_zr(xt, gorder):
                """Open z|r accumulations with the x-side contributions."""
                zr_t = [None] * G
                for g in gorder:
                    xs = xt[:, g * PG : (g + 1) * PG]
                    zr = ps.tile([C, 2 * PG], f32, tag=f"zr_{g}", bufs=1)
                    nc.tensor.matmul(
                        zr[:, PG:], wslice(WRX), xs, start=True, stop=False
                    )
                    nc.tensor.matmul(
                        zr[:, :PG], wslice(WZX), xs, start=True, stop=False
                    )
                    zr_t[g] = zr
                return zr_t

            def open_c(xt, gorder):
                cp_t = [None] * G
                for g in gorder:
                    xs = xt[:, g * PG : (g + 1) * PG]
                    cp = ps.tile([C, PG], f32, tag=f"c_{g}", bufs=2)
                    nc.tensor.matmul(
                        cp[:], wslice(WHX), xs, start=True, stop=False
                    )
                    cp_t[g] = cp
                return cp_t

            first = list(range(G))
            x_t = load_x(0)
            x_next = load_x(1)

            # -- t = 0 fast path (h == 0): only z and c are needed --
            #    z0 = sigmoid(Wzx@x0 + bz); c0 = tanh(Whx@x0 + bh);
            #    h1 = z0 * c0.  No r work, no h-closes, no blend adds.
            zr_t = [None] * G
            cp_t = [None] * G
            for g in first:
                xs = x_t[:, g * PG : (g + 1) * PG]
                zr = ps.tile([C, 2 * PG], f32, tag=f"zr_{g}", bufs=1)
                nc.tensor.matmul(
                    zr[:, :PG], wslice(WZX), xs, start=True, stop=True
                )
                zr_t[g] = zr
                cp = ps.tile([C, PG], f32, tag=f"c_{g}", bufs=2)
                nc.tensor.matmul(cp[:], wslice(WHX), xs, start=True, stop=True)
                cp_t[g] = cp

            for g in first:
                zt = wk.tile([C, PG], f16, tag=f"z_{g}")
                nc.scalar.activation(
                    zt[:], zr_t[g][:, :PG], AF.Sigmoid, bias=bt[:, 0:1]
                )
                ct = wk.tile([C, PG], f16, tag=f"c16_{g}")
                nc.scalar.activation(ct[:], cp_t[g][:], AF.Tanh, bias=bt[:, 2:3])
                n16 = spool.tile([C, PG], f16, tag=f"h16_{g}")
                nc.vector.tensor_mul(n16[:], zt[:], ct[:])
                h16[g] = n16
                nc.sync.dma_start(o_ap[0, :, g * PG : (g + 1) * PG], n16[:])

            # openers for t = 1
            x_t = x_next
            zr_t = open_zr(x_t, first)
            cp_t = open_c(x_t, first)

            for t in range(1, T):
                go = first if t % 2 == 0 else first[::-1]
                x_next = load_x(t + 1) if t + 1 < T else None

                # -- PE: close the r then z accumulations (chain head) --
                for g in go:
                    nc.tensor.matmul(
                        zr_t[g][:, PG:], wslice(WRH), h16[g][:],
                        start=False, stop=True,
                    )
                for g in go:
                    nc.tensor.matmul(
                        zr_t[g][:, :PG], wslice(WZH), h16[g][:],
                        start=False, stop=True,
                    )

                # -- ACT: r sigmoids first (they gate rh -> c matmul) --
                r16 = [None] * G
                for g in go:
                    rt = wk.tile([C, PG], f16, tag=f"r_{g}")
                    nc.scalar.activation(
                        rt[:], zr_t[g][:, PG:], AF.Sigmoid, bias=bt[:, 1:2]
                    )
                    r16[g] = rt

                rh16 = [None] * G
                for g in go:
                    rh = wk.tile([C, PG], f16, tag=f"rh_{g}")
                    nc.vector.tensor_mul(rh[:], r16[g][:], h16[g][:])
                    rh16[g] = rh

                for g in go:
                    nc.tensor.matmul(
                        cp_t[g][:], wslice(WHH), rh16[g][:],
                        start=False, stop=True,
                    )

                # next step's c openers can run any time (double-buffered)
                cp_next = open_c(x_next, go) if x_next is not None else None

                # -- ACT: zbar/tanh interleaved; zbar = 1-z = sigmoid(-pre)
                #    feeds the blend h' = zbar*h + (1-zbar)*c, whose only
                #    post-tanh serial ops are v = z*c and h' = u + v --
                zb16, c16 = [None] * G, [None] * G
                for g in go:
                    zbt = wk.tile([C, PG], f16, tag=f"zb_{g}")
                    nc.scalar.activation(
                        zbt[:], zr_t[g][:, :PG], AF.Sigmoid,
                        bias=bt[:, 3:4], scale=-1.0,
                    )
                    zb16[g] = zbt
                    ct = wk.tile([C, PG], f16, tag=f"c16_{g}")
                    nc.scalar.activation(
                        ct[:], cp_t[g][:], AF.Tanh, bias=bt[:, 2:3]
                    )
                    c16[g] = ct

                # next step's z|r openers (wait on this step's sigmoids)
                zr_next = open_zr(x_next, go) if x_next is not None else None

                # -- DVE mid-chain: u = zbar*h and z = 1-zbar overlap the
                #    tanh; only v and the final add trail it --
                u16, z16 = [None] * G, [None] * G
                for g in go:
                    ut = wk.tile([C, PG], f16, tag=f"u_{g}")
                    nc.vector.tensor_mul(ut[:], zb16[g][:], h16[g][:])
                    u16[g] = ut
                    zt = wk.tile([C, PG], f16, tag=f"z_{g}")
                    nc.vector.tensor_scalar(
                        zt[:], zb16[g][:], -1.0, 1.0,
                        mybir.AluOpType.mult, mybir.AluOpType.add,
                    )
                    z16[g] = zt

                for g in go:
                    v16 = wk.tile([C, PG], f16, tag=f"v_{g}")
                    nc.vector.tensor_mul(v16[:], z16[g][:], c16[g][:])
                    n16 = spool.tile([C, PG], f16, tag=f"h16_{g}")
                    nc.vector.tensor_add(n16[:], u16[g][:], v16[:])
                    h16[g] = n16
                    nc.sync.dma_start(
                        o_ap[t, :, g * PG : (g + 1) * PG], n16[:]
                    )

                if x_next is not None:
                    x_t, zr_t, cp_t = x_next, zr_next, cp_next

    nc.compile()
    return nc


def _get_prog():
    global _PROG
    if _PROG is None:
        _PROG = _build()
    return _PROG


def _make_in_maps(video, Wz, bz, Wr, br, Wh, bh):
    w6 = np.concatenate(
        [
            Wz[:, :C].T, Wz[:, C:].T,
            Wr[:, :C].T, Wr[:, C:].T,
            Wh[:, :C].T, Wh[:, C:].T,
        ],
        axis=1,
    ).astype(np.float16)
    b3 = np.stack([bz, br, bh, -bz], axis=1).astype(np.float32)
    in_maps = []
    for core in range(NCORES):
        b_, q = divmod(core, 4)
        xs = np.ascontiguousarray(
            video[b_, :, :, q * HQ : (q + 1) * HQ, :]
        ).reshape(T, C, P).astype(np.float16)
        in_maps.append({"x_seq": xs, "wmats": w6, "biases": b3})
    return in_maps


def kernel(video, Wz, bz, Wr, br, Wh, bh):
    _ensure_paths()
    from concourse.bass_utils import run_bass_kernel_spmd

    video = np.asarray(video, dtype=np.float32)
    nc = _get_prog()
    in_maps = _make_in_maps(video, Wz, bz, Wr, br, Wh, bh)
    res = run_bass_kernel_spmd(nc, in_maps, list(range(NCORES)))

    out = np.empty((B, T, C, H, W), np.float32)
    for core in range(NCORES):
        b_, q = divmod(core, 4)
        out[b_, :, :, q * HQ : (q + 1) * HQ, :] = np.asarray(
            res.results[core]["out_seq"]
        ).astype(np.float32).reshape(T, C, HQ, W)
    return out



# revision 31
# speedup vs baseline: 1.2403x; 1.0321x over previous
"""ConvGRU Trainium2 kernel.

video [B=2, T=16, C=128, H=64, W=64] f32; 1x1-conv GRU over T.
Sharding: data-parallel over (B x H/16) -> 8 cores, each core owns
P = 16*64 = 1024 pixels for all T; weights replicated.

Per core, per timestep (pixels on the free dim, channels on partitions):
    zr_pre = [Wzx@x + Wzh@h | Wrx@x + Wrh@h]      (PE, fp16 in / fp32 psum)
    z = sigmoid(zr_pre[:P] + bz); r = sigmoid(zr_pre[P:] + br)   (ACT)
    rh = r * h                                     (DVE)
    c = tanh(Whx@x + Whh@rh + bh)                  (PE + ACT)
    h = h + z * (c - h)                            (DVE, fp16 state)

The recurrence is latency-bound: each pixel group's step is a serial
cross-engine chain (h -> Wrh matmul -> sigmoid -> r*h -> Whh matmul ->
tanh -> blend -> h').  Structure choices below all serve that chain:
  - x-side matmul contributions for step t+1 are issued into step t's
    tail (PSUM c-tiles double buffered) to keep the PE dense/warm
  - r-gate work goes first (it gates the tanh matmul); z sigmoids are
    slotted late (only needed by the final blend)
  - group priority alternates per step so the second group's queueing
    penalty averages out instead of compounding on one chain
  - warmup matmuls + an early dummy activation hide the HAM clock-gate
    ramp and the ACT table load behind the initial x DMA

Numerics: fp16 matmul inputs/gates/state, fp32 PSUM accum + fp32 bias.
"""

import os
import sys

import numpy as np

B, T, C, H, W = 2, 16, 128, 64, 64
NCORES = 8
HQ = H // 4          # 16 rows of H per core (4 H-slices x 2 batches = 8 cores)
P = HQ * W           # 1024 pixels per core
G = 2                # pixel groups per step (independent recurrence chains)
PG = P // G          # 512 pixels per group

_PROG = None


def _ensure_paths():
    for p in ("/opt/trn_rl_repo",):
        if p not in sys.path and os.path.isdir(p):
            sys.path.append(p)


def _build():
    _ensure_paths()
    import concourse.bacc as bacc
    import concourse.tile as tile
    from concourse import mybir

    f32 = mybir.dt.float32
    f16 = mybir.dt.float16
    AF = mybir.ActivationFunctionType

    nc = bacc.Bacc(
        "TRN2", target_bir_lowering=False, debug=False, num_devices=NCORES
    )
    x_dram = nc.dram_tensor("x_seq", [T, C, P], f16, kind="ExternalInput")
    w_dram = nc.dram_tensor("wmats", [C, 6 * C], f16, kind="ExternalInput")
    b_dram = nc.dram_tensor("biases", [C, 4], f32, kind="ExternalInput")
    o_dram = nc.dram_tensor("out_seq", [T, C, P], f16, kind="ExternalOutput")

    x_ap = x_dram.ap()
    w_ap = w_dram.ap()
    b_ap = b_dram.ap()
    o_ap = o_dram.ap()

    WZX, WZH, WRX, WRH, WHX, WHH = range(6)

    with tile.TileContext(nc) as tc:
        with (
            tc.tile_pool(name="consts", bufs=1) as consts,
            tc.tile_pool(name="xin", bufs=4) as xpool,
            tc.tile_pool(name="state", bufs=2) as spool,
            tc.tile_pool(name="work", bufs=2) as wk,
            tc.tile_pool(name="ps", bufs=1, space="PSUM") as ps,
        ):
            # garbage tile: warmup matmuls need no DMA, so the PE ramp and
            # ACT table load start immediately and overlap the input DMAs
            junk = consts.tile([C, PG], f16)
            nc.vector.memset(junk[:], 0.0)

            wt = consts.tile([C, 6 * C], f16)
            nc.sync.dma_start(wt[:], w_ap[:])
            bt = consts.tile([C, 4], f32)
            nc.gpsimd.dma_start(bt[:], b_ap[:])

            def wslice(i):
                return wt[:, i * C : (i + 1) * C]

            h16 = [None] * G

            # -- warmup: ramp the PE clock gate + preload the ACT table
            #    while the first x DMA is in flight.  Aliases the c_0 tag
            #    (bufs=2) so t0's single-buffered zr openers don't WAW-wait
            #    on the warmup sigmoid's read. --
            warm = ps.tile([C, PG], f32, tag="c_0", bufs=2)
            for i in range(6):
                nc.tensor.matmul(
                    warm[:], junk[:, :C], junk[:], start=True, stop=True
                )
            wtmp = wk.tile([C, PG], f16, tag="r_0")
            nc.scalar.activation(
                wtmp[:, :C], warm[:, :C], AF.Sigmoid, bias=0.0
            )

            def load_x(t):
                xt = xpool.tile([C, P], f16, tag="x")
                nc.sync.dma_start(xt[:], x_ap[t])
                return xt

            def open_zr(xt, gorder):
                """Open z|r accumulations with the x-side contributions."""
                zr_t = [None] * G
                for g in gorder:
                    xs = xt[:, g * PG : (g + 1) * PG]
                    zr = ps.tile([C, 2 * PG], f32, tag=f"zr_{g}", bufs=1)
                    nc.tensor.matmul(
                        zr[:, PG:], wslice(WRX), xs, start=True, stop=False
                    )
                    nc.tensor.matmul(
                        zr[:, :PG], wslice(WZX), xs, start=True, stop=False
                    )
                    zr_t[g] = zr
                return zr_t

            def open_c(xt, gorder):
                cp_t = [None] * G
                for g in gorder:
                    xs = xt[:, g * PG : (g + 1) * PG]
                    cp = ps.tile([C, PG], f32, tag=f"c_{g}", bufs=2)
                    nc.tensor.matmul(
                        cp[:], wslice(WHX), xs, start=True, stop=False
                    )
                    cp_t[g] = cp
                return cp_t

            first = list(range(G))
            x_t = load_x(0)
            x_next = load_x(1)

            # -- t = 0 fast path (h == 0): only z and c are needed --
            #    z0 = sigmoid(Wzx@x0 + bz); c0 = tanh(Whx@x0 + bh);
            #    h1 = z0 * c0.  No r work, no h-closes, no blend adds.
            zr_t = [None] * G
            cp_t = [None] * G
            for g in first:
                xs = x_t[:, g * PG : (g + 1) * PG]
                zr = ps.tile([C, 2 * PG], f32, tag=f"zr_{g}", bufs=1)
                nc.tensor.matmul(
                    zr[:, :PG], wslice(WZX), xs, start=True, stop=True
                )
                zr_t[g] = zr
                cp = ps.tile([C, PG], f32, tag=f"c_{g}", bufs=2)
                nc.tensor.matmul(cp[:], wslice(WHX), xs, start=True, stop=True)
                cp_t[g] = cp

            for g in first:
                zt = wk.tile([C, PG], f16, tag=f"z_{g}")
                nc.scalar.activation(
                    zt[:], zr_t[g][:, :PG], AF.Sigmoid, bias=bt[:, 0:1]
                )
                ct = wk.tile([C, PG], f16, tag=f"c16_{g}")
                nc.scalar.activation(ct[:], cp_t[g][:], AF.Tanh, bias=bt[:, 2:3])
                n16 = spool.tile([C, PG], f16, tag=f"h16_{g}")
                nc.vector.tensor_mul(n16[:], zt[:], ct[:])
                h16[g] = n16
                nc.sync.dma_start(o_ap[0, :, g * PG : (g + 1) * PG], n16[:])

            # openers for t = 1
            x_t = x_next
            zr_t = open_zr(x_t, first)
            cp_t = open_c(x_t, first)

            for t in range(1, T):
                go = first if t % 2 == 0 else first[::-1]
                x_next = load_x(t + 1) if t + 1 < T else None

                # -- PE: close the r then z accumulations (chain head) --
                for g in go:
                    nc.tensor.matmul(
                        zr_t[g][:, PG:], wslice(WRH), h16[g][:],
                        start=False, stop=True,
                    )
                for g in go:
                    nc.tensor.matmul(
                        zr_t[g][:, :PG], wslice(WZH), h16[g][:],
                        start=False, stop=True,
                    )

                # -- ACT: r sigmoids first (they gate rh -> c matmul) --
                r16 = [None] * G
                for g in go:
                    rt = wk.tile([C, PG], f16, tag=f"r_{g}")
                    nc.scalar.activation(
                        rt[:], zr_t[g][:, PG:], AF.Sigmoid, bias=bt[:, 1:2]
                    )
                    r16[g] = rt

                rh16 = [None] * G
                for g in go:
                    rh = wk.tile([C, PG], f16, tag=f"rh_{g}")
                    nc.vector.tensor_mul(rh[:], r16[g][:], h16[g][:])
                    rh16[g] = rh

                for g in go:
                    nc.tensor.matmul(
                        cp_t[g][:], wslice(WHH), rh16[g][:],
                        start=False, stop=True,
                    )

                # next step's c openers can run any time (double-buffered)
                cp_next = open_c(x_next, go) if x_next is not None else None

                # -- ACT: zbar/tanh interleaved; zbar = 1-z = sigmoid(-pre)
                #    feeds the blend h' = zbar*h + (1-zbar)*c, whose only
                #    post-tanh serial ops are v = z*c and h' = u + v --
                zb16, c16 = [None] * G, [None] * G
                for g in go:
                    zbt = wk.tile([C, PG], f16, tag=f"zb_{g}")
                    nc.scalar.activation(
                        zbt[:], zr_t[g][:, :PG], AF.Sigmoid,
                        bias=bt[:, 3:4], scale=-1.0,
                    )
                    zb16[g] = zbt
                    ct = wk.tile([C, PG], f16, tag=f"c16_{g}")
                    nc.scalar.activation(
                        ct[:], cp_t[g][:], AF.Tanh, bias=bt[:, 2:3]
                    )
                    c16[g] = ct

                # next step's z|r openers (wait on this step's sigmoids)
                zr_next = open_zr(x_next, go) if x_next is not None else None

                # -- DVE mid-chain: u = zbar*h and z = 1-zbar overlap the
                #    tanh; only v and the final add trail it --
                u16, z16 = [None] * G, [None] * G
                for g in go:
                    ut = wk.tile([C, PG], f16, tag=f"u_{g}")
                    nc.vector.tensor_mul(ut[:], zb16[g][:], h16[g][:])
                    u16[g] = ut
                    zt = wk.tile([C, PG], f16, tag=f"z_{g}")
                    nc.vector.tensor_scalar(
                        zt[:], zb16[g][:], -1.0, 1.0,
                        mybir.AluOpType.mult, mybir.AluOpType.add,
                    )
                    z16[g] = zt

                for g in go:
                    v16 = wk.tile([C, PG], f16, tag=f"v_{g}")
                    nc.vector.tensor_mul(v16[:], z16[g][:], c16[g][:])
                    n16 = spool.tile([C, PG], f16, tag=f"h16_{g}")
                    nc.vector.tensor_add(n16[:], u16[g][:], v16[:])
                    h16[g] = n16
                    nc.sync.dma_start(
                        o_ap[t, :, g * PG : (g + 1) * PG], n16[:]
                    )

                if x_next is not None:
                    x_t, zr_t, cp_t = x_next, zr_next, cp_next

    nc.compile()
    return nc


def _get_prog():
    global _PROG
    if _PROG is None:
        _PROG = _build()
    return _PROG


def _make_in_maps(video, Wz, bz, Wr, br, Wh, bh):
    w6 = np.concatenate(
        [
            Wz[:, :C].T, Wz[:, C:].T,
            Wr[:, :C].T, Wr[:, C:].T,
            Wh[:, :C].T, Wh[:, C:].T,
        ],
        axis=1,
    ).astype(np.float16)
    b3 = np.stack([bz, br, bh, -bz], axis=1).astype(np.float32)
    in_maps = []
    for core in range(NCORES):
        b_, q = divmod(core, 4)
        xs = np.ascontiguousarray(
            video[b_, :, :, q * HQ : (q + 1) * HQ, :]
        ).reshape(T, C, P).astype(np.float16)
        in_maps.append({"x_seq": xs, "wmats": w6, "biases": b3})
    return in_maps


def kernel(video, Wz, bz, Wr, br, Wh, bh):
    _ensure_paths()
    from concourse.bass_utils import run_bass_kernel_spmd

    video = np.asarray(video, dtype=np.float32)
    nc = _get_prog()
    in_maps = _make_in_maps(video, Wz, bz, Wr, br, Wh, bh)
    res = run_bass_kernel_spmd(nc, in_maps, list(range(NCORES)))

    out = np.empty((B, T, C, H, W), np.float32)
    for core in range(NCORES):
        b_, q = divmod(core, 4)
        out[b_, :, :, q * HQ : (q + 1) * HQ, :] = np.asarray(
            res.results[core]["out_seq"]
        ).astype(np.float32).reshape(T, C, HQ, W)
    return out

